# revision 2
# baseline (speedup 1.0000x reference)
import sys
sys.path.insert(0, '/opt/trn_rl_repo')
import numpy as np
import ml_dtypes

import concourse.bass as bass
import concourse.tile as tile
from concourse import bacc, mybir
from concourse.bass_utils import run_bass_kernel_spmd
from concourse.masks import make_identity

DIM = 2048
BSZ, SEQ = 2, 2048
S = SEQ
THRESHOLD = 0.05
HPC = 8                      # q heads per core
KVPC = 2                     # kv heads per core
NPAIR = 4                    # q-head pairs per core
SB = 512
NSB = S // SB                # 4
NDC = DIM // 128             # 16 contraction chunks
NQT = S // 128               # 16 q tiles

f32 = mybir.dt.float32
f32r = mybir.dt.float32r
bf16 = mybir.dt.bfloat16
bf = ml_dtypes.bfloat16
EXP = mybir.ActivationFunctionType.Exp
AX = mybir.AxisListType.X
MAXOP = mybir.AluOpType.max
MINOP = mybir.AluOpType.min
ADDOP = mybir.AluOpType.add


def _ternarize(w):
    w = w.astype(np.float64)
    scale = max(np.abs(w).mean(), 1e-6)
    return np.where(w > THRESHOLD * scale, 1.0,
                    np.where(w < -THRESHOLD * scale, -1.0, 0.0))


def build_program():
    nc = bacc.Bacc(None, target_bir_lowering=False, debug=False)

    def din(name, shape, dt):
        return nc.dram_tensor(name, list(shape), dt, kind="ExternalInput").ap()

    xT_d = din("xT", (DIM, S), f32)          # x[b].T fp32
    wq_d = din("wq", (DIM, 512), bf16)       # ternary(wq).T/8 cols (8 heads), exact bf16
    wk_d = din("wk", (DIM, 128), bf16)       # ternary(wk).T cols (2 kv heads)
    wv_d = din("wv", (DIM, 128), bf16)
    wo_d = din("wo", (512, DIM), bf16)       # ternary(wo).T rows = this core's feats
    tri_d = din("tri", (128, 128), f32)      # strictly-upper -1e30, else 0
    oT_d = nc.dram_tensor("oT", [DIM, S], bf16, kind="ExternalOutput").ap()

    with tile.TileContext(nc) as tc:
        with tc.tile_pool(name="persist", bufs=1) as pp:
            tri = pp.tile([128, 128], f32)
            nc.sync.dma_start(tri[:], tri_d[:])
            identb = pp.tile([128, 128], bf16)
            make_identity(nc, identb[:])

            # persistent activations
            qt = [pp.tile([128, S], f32, tag=f"qt{m}", name=f"qt{m}") for m in range(NPAIR)]
            kk = [pp.tile([64, S], f32, tag=f"kk{v}", name=f"kk{v}") for v in range(KVPC)]
            # va: [s-part, chunk, kv, 64 v-feats + ones]
            va = pp.tile([128, NDC, KVPC, 65], bf16)
            nc.vector.memset(va[:, :, :, 64:65], 1.0)
            # normalized attention outputs (q-major) awaiting transpose
            nout = [pp.tile([128, 2, NQT, 64], bf16, tag=f"no{hp}", name=f"no{hp}")
                    for hp in range(NPAIR)]
            # o-proj rhs: per head-pair features x seq (bf16)
            ot = [pp.tile([128, S], bf16, tag=f"ot{hp}", name=f"ot{hp}") for hp in range(NPAIR)]

            # ---------------- phase 1: projections (all f32r) ----------------
            with tc.tile_pool(name="w1", bufs=1) as wp, \
                 tc.tile_pool(name="xp", bufs=4) as xp, \
                 tc.tile_pool(name="ps1", bufs=1, space="PSUM") as ps1:
                wq_b = wp.tile([128, NDC, 512], bf16)
                wk_b = wp.tile([128, NDC, 128], bf16)
                wv_b = wp.tile([128, NDC, 128], bf16)
                wq_f = wp.tile([128, NDC, 512], f32)
                wk_f = wp.tile([128, NDC, 128], f32)
                wv_f = wp.tile([128, NDC, 128], f32)
                vf = wp.tile([128, S], bf16)     # v feat-major staging
                for dc in range(NDC):
                    dsl = bass.ds(dc * 128, 128)
                    nc.sync.dma_start(wq_b[:, dc, :], wq_d[dsl, :])
                    nc.sync.dma_start(wk_b[:, dc, :], wk_d[dsl, :])
                    nc.sync.dma_start(wv_b[:, dc, :], wv_d[dsl, :])
                    nc.vector.tensor_copy(wq_f[:, dc, :], wq_b[:, dc, :])
                    nc.scalar.copy(wk_f[:, dc, :], wk_b[:, dc, :])
                    nc.scalar.copy(wv_f[:, dc, :], wv_b[:, dc, :])

                for sb_i in range(NSB):
                    ssl = bass.ts(sb_i, SB)
                    ps_q = [ps1.tile([128, SB], f32, tag=f"psq{m}", name=f"psq{m}")
                            for m in range(NPAIR)]
                    ps_k = ps1.tile([128, SB], f32, tag="psk")
                    ps_v = ps1.tile([128, SB], f32, tag="psv")
                    for dc in range(NDC):
                        xt = xp.tile([128, SB], f32, tag="x")
                        nc.sync.dma_start(xt[:], xT_d[bass.ds(dc * 128, 128), ssl])
                        st = (dc == 0)
                        sp = (dc == NDC - 1)
                        xr = xt[:].bitcast(f32r)
                        for m in range(NPAIR):
                            nc.tensor.matmul(ps_q[m][:],
                                             wq_f[:, dc, bass.ts(m, 128)].bitcast(f32r),
                                             xr, start=st, stop=sp)
                        nc.tensor.matmul(ps_k[:], wk_f[:, dc, :].bitcast(f32r),
                                         xr, start=st, stop=sp)
                        nc.tensor.matmul(ps_v[:], wv_f[:, dc, :].bitcast(f32r),
                                         xr, start=st, stop=sp)
                    # evacuate
                    for m in range(NPAIR):
                        if m % 2 == 0:
                            nc.vector.tensor_copy(qt[m][:, ssl], ps_q[m][:])
                        else:
                            nc.scalar.copy(qt[m][:, ssl], ps_q[m][:])
                    for v in range(KVPC):
                        nc.vector.tensor_copy(kk[v][:, ssl], ps_k[bass.ds(v * 64, 64), :])
                    nc.scalar.copy(vf[:, ssl], ps_v[:])
                # v: feat-major -> s-major chunks via DMA transpose
                for c in range(NDC):
                    nc.sync.dma_start_transpose(va[:, c, :, 0:64],
                                                vf[:, bass.ts(c, 128)])

            # ---------------- phase 2: attention ----------------
            with tc.tile_pool(name="sps", bufs=1, space="PSUM") as sps, \
                 tc.tile_pool(name="pvp", bufs=1, space="PSUM") as pvp, \
                 tc.tile_pool(name="ptp", bufs=3) as ptp, \
                 tc.tile_pool(name="ptTp", bufs=3) as ptTp, \
                 tc.tile_pool(name="stp", bufs=4) as stp:
                for hp in range(NPAIR):
                    kv = hp // 2
                    for h in range(2):
                        hsl = bass.ds(h * 64, 64)
                        for qi in range(NQT):
                            nk = qi // 4 + 1
                            qsl = bass.ts(qi, 128)
                            lhs_q = qt[hp][hsl, qsl].bitcast(f32r)
                            nmx = stp.tile([128, 4], f32, tag="nmx")
                            sblk = []
                            for kb in range(nk):
                                kw = 512 if kb < nk - 1 else 128 * (qi % 4 + 1)
                                s0 = sps.tile([128, SB], f32, tag="s", bufs=6,
                                              name=f"s{kb}")
                                sblk.append((s0, kw))
                                nc.tensor.matmul(
                                    s0[:, 0:kw], lhs_q,
                                    kk[kv][:, bass.ds(kb * 512, kw)].bitcast(f32r),
                                    start=True, stop=True, tile_position=(0, 0))
                                if kb == nk - 1:
                                    nc.gpsimd.tensor_tensor(
                                        s0[:, kw - 128:kw], s0[:, kw - 128:kw],
                                        tri[:], ADDOP)
                                nc.vector.tensor_reduce(
                                    nmx[:, kb:kb + 1], s0[:, 0:kw], AX, MAXOP,
                                    negate=True)
                            negmax = stp.tile([128, 1], f32, tag="ngm")
                            nc.vector.tensor_reduce(negmax[:], nmx[:, 0:nk], AX, MINOP)
                            # exp into q-major staging
                            p_t = ptp.tile([128, S], bf16, tag="p")
                            for kb, (s0, kw) in enumerate(sblk):
                                nc.scalar.activation(
                                    p_t[:, bass.ds(kb * 512, kw)], s0[:, 0:kw],
                                    EXP, bias=negmax[:], scale=1.0)
                            # transpose each 128-chunk via DMA xbar
                            ptT = ptTp.tile([128, NQT, 128], bf16, tag="ptT")
                            for c in range(qi + 1):
                                nc.sync.dma_start_transpose(
                                    ptT[:, c, :], p_t[:, bass.ts(c, 128)])
                            # PV: out [128 q, 65]
                            pvq = pvp.tile([128, 65], f32, tag="pv", bufs=2)
                            for c in range(qi + 1):
                                nc.tensor.matmul(
                                    pvq[:], ptT[:, c, :], va[:, c, kv, :],
                                    start=(c == 0), stop=(c == qi))
                            rr = stp.tile([128, 1], f32, tag="rr")
                            nc.vector.reciprocal(rr[:], pvq[:, 64:65])
                            nc.scalar.mul(nout[hp][:, h, qi, :], pvq[:, 0:64], rr[:])

            # ---------------- phase 2.5: transpose nout -> ot ----------------
            with tc.tile_pool(name="ops", bufs=1, space="PSUM") as ops:
                for hp in range(NPAIR):
                    for h in range(2):
                        for qg in range(4):
                            otp = ops.tile([64, 512], bf16, tag="otp", bufs=2)
                            for j in range(4):
                                qi = qg * 4 + j
                                nc.tensor.matmul(
                                    otp[:, bass.ts(j, 128)],
                                    nout[hp][:, h, qi, :], identb[:],
                                    is_transpose=True,
                                    start=(j == 0), stop=(j == 3))
                            nc.vector.tensor_copy(
                                ot[hp][bass.ds(h * 64, 64), bass.ts(qg, 512)], otp[:])

            # ---------------- phase 3: output projection (bf16) ----------------
            with tc.tile_pool(name="w3", bufs=1) as w3p, \
                 tc.tile_pool(name="ps3", bufs=1, space="PSUM") as ps3, \
                 tc.tile_pool(name="ev3", bufs=4) as ev3:
                wo_b = w3p.tile([128, 4, DIM], bf16)
                for fc in range(4):
                    nc.sync.dma_start(wo_b[:, fc, :], wo_d[bass.ds(fc * 128, 128), :])
                for mo in range(16):
                    for sb_i in range(NSB):
                        pso = ps3.tile([128, SB], f32, tag="pso", bufs=4)
                        for fc in range(4):
                            nc.tensor.matmul(
                                pso[:], wo_b[:, fc, bass.ts(mo, 128)],
                                ot[fc][:, bass.ts(sb_i, SB)],
                                start=(fc == 0), stop=(fc == 3))
                        ob = ev3.tile([128, SB], bf16, tag="ob")
                        if (mo + sb_i) % 2 == 0:
                            nc.vector.tensor_copy(ob[:], pso[:])
                        else:
                            nc.scalar.copy(ob[:], pso[:])
                        nc.sync.dma_start(
                            oT_d[bass.ts(mo, 128), bass.ts(sb_i, SB)], ob[:])

    nc.compile()
    return nc


_PROG = None


def kernel(x, wq, wk, wv, wo):
    global _PROG
    if _PROG is None:
        _PROG = build_program()
    nc = _PROG

    twq = _ternarize(wq) / 8.0          # fold softmax scale into q
    twk = _ternarize(wk)
    twv = _ternarize(wv)
    two = _ternarize(wo)
    tri_np = (np.triu(np.ones((128, 128), np.float64), 1) * -1e30).astype(np.float32)

    xT = [np.ascontiguousarray(x[b].astype(np.float32).T) for b in range(BSZ)]
    in_maps = []
    for c in range(8):
        b, hq = c % 2, c // 2
        qcols = slice(hq * 512, (hq + 1) * 512)
        kvcols = slice(hq * 128, (hq + 1) * 128)
        in_maps.append({
            "xT": xT[b],
            "wq": np.ascontiguousarray(twq.T[:, qcols]).astype(bf),
            "wk": np.ascontiguousarray(twk.T[:, kvcols]).astype(bf),
            "wv": np.ascontiguousarray(twv.T[:, kvcols]).astype(bf),
            "wo": np.ascontiguousarray(two.T[hq * 512:(hq + 1) * 512, :]).astype(bf),
            "tri": tri_np,
        })

    res = run_bass_kernel_spmd(nc, in_maps, list(range(8)))

    out = np.zeros((BSZ, SEQ, DIM), np.float32)
    for c in range(8):
        b = c % 2
        out[b] += res.results[c]["oT"].astype(np.float32).T
    return out


# revision 15
# speedup vs baseline: 1.7290x; 1.7290x over previous
import sys
sys.path.insert(0, '/opt/trn_rl_repo')
import numpy as np
import ml_dtypes

import concourse.bass as bass
import concourse.tile as tile
from concourse import bacc, mybir
from concourse.bass_utils import run_bass_kernel_spmd
from concourse.masks import make_identity

DIM = 2048
BSZ, SEQ = 2, 2048
S = SEQ
THRESHOLD = 0.05
HPC = 8                      # q heads per core
KVPC = 2                     # kv heads per core
NPAIR = 4                    # q-head pairs per core
SB = 512
NSB = S // SB                # 4
NDC = DIM // 128             # 16 contraction chunks
NQT = S // 128               # 16 q tiles

f32 = mybir.dt.float32
f32r = mybir.dt.float32r
bf16 = mybir.dt.bfloat16
bf = ml_dtypes.bfloat16
EXP = mybir.ActivationFunctionType.Exp
AX = mybir.AxisListType.X
MAXOP = mybir.AluOpType.max
MINOP = mybir.AluOpType.min
ADDOP = mybir.AluOpType.add

ROW_LAG = 2                  # rows between scores and transpose/PV consumption


def _ternarize(w):
    w = w.astype(np.float64)
    scale = max(np.abs(w).mean(), 1e-6)
    return np.where(w > THRESHOLD * scale, 1.0,
                    np.where(w < -THRESHOLD * scale, -1.0, 0.0))


def build_program():
    nc = bacc.Bacc(None, target_bir_lowering=False, debug=False)

    def din(name, shape, dt):
        return nc.dram_tensor(name, list(shape), dt, kind="ExternalInput").ap()

    xT_d = din("xT", (DIM, S), f32)          # x[b].T fp32
    wq_d = din("wq", (DIM, 512), f32)        # ternary(wq).T/8 cols (8 heads)
    wk_d = din("wk", (DIM, 128), f32)
    wv_d = din("wv", (DIM, 128), f32)
    wo_d = din("wo", (512, DIM), bf16)       # ternary(wo).T rows = core's feats
    tri_d = din("tri", (128, 128), f32)      # strictly-upper -1e30, else 0
    oT_d = nc.dram_tensor("oT", [DIM, S], bf16, kind="ExternalOutput").ap()

    with tile.TileContext(nc) as tc:
        with tc.tile_pool(name="persist", bufs=1) as pp, \
             tc.tile_pool(name="wts", bufs=1) as wp, \
             tc.tile_pool(name="xq", bufs=3) as xqp, \
             tc.tile_pool(name="vfp", bufs=2) as vfp, \
             tc.tile_pool(name="ptp", bufs=3) as ptp, \
             tc.tile_pool(name="ptTp", bufs=2) as ptTp, \
             tc.tile_pool(name="stp", bufs=6) as stp, \
             tc.tile_pool(name="otp", bufs=2) as otpool, \
             tc.tile_pool(name="nop", bufs=2) as nopool, \
             tc.tile_pool(name="obp", bufs=1) as obp, \
             tc.tile_pool(name="acc", bufs=3, space="PSUM") as accp, \
             tc.tile_pool(name="sps", bufs=4, space="PSUM") as sps, \
             tc.tile_pool(name="tpp", bufs=1, space="PSUM") as tpp:

            tri = pp.tile([128, 128], f32)
            nc.sync.dma_start(tri[:], tri_d[:])
            identb = pp.tile([128, 128], bf16)
            make_identity(nc, identb[:])

            qt = [pp.tile([128, S], f32, tag=f"qt{m}", name=f"qt{m}") for m in range(NPAIR)]
            kk = [pp.tile([64, S], f32, tag=f"kk{v}", name=f"kk{v}") for v in range(KVPC)]
            va = pp.tile([128, NDC, KVPC, 65], bf16)
            nc.vector.memset(va[:, :, :, 64:65], 1.0)

            # weights resident all run
            wq_f = wp.tile([128, NDC, 512], f32)
            wk_f = wp.tile([128, NDC, 128], f32)
            wv_f = wp.tile([128, NDC, 128], f32)
            wo_b = wp.tile([128, 4, DIM], bf16)
            nc.sync.dma_start(
                wq_f[:], wq_d[:, :].rearrange("(a p) b -> p a b", p=128))
            nc.sync.dma_start(
                wk_f[:], wk_d[:, :].rearrange("(a p) b -> p a b", p=128))
            nc.sync.dma_start(
                wv_f[:], wv_d[:, :].rearrange("(a p) b -> p a b", p=128))
            nc.sync.dma_start(
                wo_b[:], wo_d[:, :].rearrange("(a p) b -> p a b", p=128))

            # ---------- emission helpers ----------
            def emit_x_dma(sb_i, g):
                """load dc quad g (4 chunks) of x for seq-block sb_i"""
                xt = xqp.tile([128, 4, SB], f32, tag="x", name="xt")
                nc.sync.dma_start(
                    xt[:],
                    xT_d[g * 512:(g + 1) * 512, bass.ts(sb_i, SB)].rearrange(
                        "(a p) b -> p a b", p=128))
                return xt

            def emit_proj_pass(sb_i, which):
                """which=0: Q01+K ; which=1: Q23+V.  Returns evac closures."""
                ssl = bass.ts(sb_i, SB)
                a0 = accp.tile([128, SB], f32, tag="acc", name="a0")
                a1 = accp.tile([128, SB], f32, tag="acc", name="a1")
                a2 = accp.tile([128, SB], f32, tag="acc", name="a2")
                xt = None
                for dc in range(NDC):
                    if dc % 4 == 0:
                        xt = emit_x_dma(sb_i, dc // 4)
                    xr = xt[:, dc % 4, :].bitcast(f32r)
                    st = (dc == 0)
                    sp = (dc == NDC - 1)
                    m0, m1 = (0, 1) if which == 0 else (2, 3)
                    nc.tensor.matmul(a0[:], wq_f[:, dc, bass.ts(m0, 128)].bitcast(f32r),
                                     xr, start=st, stop=sp)
                    nc.tensor.matmul(a1[:], wq_f[:, dc, bass.ts(m1, 128)].bitcast(f32r),
                                     xr, start=st, stop=sp)
                    wkv = wk_f if which == 0 else wv_f
                    nc.tensor.matmul(a2[:], wkv[:, dc, :].bitcast(f32r),
                                     xr, start=st, stop=sp)
                # evacuations
                if which == 0:
                    nc.vector.tensor_copy(qt[0][:, ssl], a0[:])
                    nc.scalar.copy(qt[1][:, ssl], a1[:])
                    for v in range(KVPC):
                        nc.vector.tensor_copy(kk[v][:, ssl], a2[bass.ds(v * 64, 64), :])
                else:
                    nc.vector.tensor_copy(qt[2][:, ssl], a0[:])
                    nc.scalar.copy(qt[3][:, ssl], a1[:])
                    vf = vfp.tile([128, SB], bf16, tag="vf")
                    nc.scalar.copy(vf[:], a2[:])
                    for j in range(4):
                        c = sb_i * 4 + j
                        nc.sync.dma_start_transpose(
                            va[:, c, :, 0:64], vf[:, bass.ts(j, 128)])

            def emit_scores(row):
                """scores + max + exp for one attention row. Returns state."""
                hp, h, qi = row
                kv = hp // 2
                nk = qi // 4 + 1
                qsl = bass.ts(qi, 128)
                lhs_q = qt[hp][bass.ds(h * 64, 64), qsl].bitcast(f32r)
                nmx = stp.tile([128, 4], f32, tag="nmx")
                sblk = []
                for kb in range(nk):
                    kw = 512 if kb < nk - 1 else 128 * (qi % 4 + 1)
                    s0 = sps.tile([128, SB], f32, tag="s", name=f"s{kb}")
                    sblk.append((s0, kw))
                    nc.tensor.matmul(
                        s0[:, 0:kw], lhs_q,
                        kk[kv][:, bass.ds(kb * 512, kw)].bitcast(f32r),
                        start=True, stop=True, tile_position=(0, 0))
                    if kb == nk - 1:
                        nc.gpsimd.tensor_tensor(
                            s0[:, kw - 128:kw], s0[:, kw - 128:kw], tri[:], ADDOP)
                    nc.vector.tensor_reduce(
                        nmx[:, kb:kb + 1], s0[:, 0:kw], AX, MAXOP, negate=True)
                negmax = stp.tile([128, 1], f32, tag="ngm")
                nc.vector.tensor_reduce(negmax[:], nmx[:, 0:nk], AX, MINOP)
                p_t = ptp.tile([128, S], bf16, tag="p")
                for kb, (s0, kw) in enumerate(sblk):
                    nc.scalar.activation(
                        p_t[:, bass.ds(kb * 512, kw)], s0[:, 0:kw],
                        EXP, bias=negmax[:], scale=1.0)
                return p_t

            copy_rr = [0]

            def emit_pv_chunks(pvst, lo, hi):
                """emit PV matmul chunks [lo, hi) for a row's pv state"""
                row, ptT, pvq = pvst
                hp, h, qi = row
                kv = hp // 2
                nch = qi + 1
                for c in range(lo, min(hi, nch)):
                    nc.tensor.matmul(pvq[:], ptT[:, c, :], va[:, c, kv, :],
                                     start=(c == 0), stop=(c == nch - 1),
                                     skip_group_check=True)

            def emit_pv_finish(pvst, nout_t):
                row, ptT, pvq = pvst
                hp, h, qi = row
                rr = stp.tile([128, 1], f32, tag="rr")
                nc.vector.reciprocal(rr[:], pvq[:, 64:65])
                nc.scalar.mul(nout_t[hp][:, h, qi % 4, :], pvq[:, 0:64], rr[:])

            def emit_transpose(row, p_t, prev_pvst):
                """transpose P chunks to ptT; weave prev row's PV between
                groups.  Returns (row, ptT, pvq) PV-state for this row."""
                hp, h, qi = row
                nch = qi + 1
                prev_nch = prev_pvst[0][2] + 1 if prev_pvst else 0
                ngroups = (nch + 3) // 4
                pv_per_gap = (prev_nch + ngroups - 1) // ngroups if prev_pvst else 0
                ptT = ptTp.tile([128, NQT, 128], bf16, tag="ptT")
                c = 0
                g = 0
                while c < nch:
                    jn = min(4, nch - c)
                    tp = tpp.tile([128, SB], bf16, tag="tp")
                    for j in range(jn):
                        nc.tensor.matmul(
                            tp[:, bass.ts(j, 128)],
                            p_t[:, bass.ts(c + j, 128)], identb[:],
                            is_transpose=True, start=(j == 0), stop=(j == jn - 1))
                    dst = ptT[:, c:c + jn, :].rearrange("p a b -> p (a b)")
                    src = tp[:, 0:jn * 128]
                    r = copy_rr[0] % 3
                    copy_rr[0] += 1
                    if r == 0:
                        nc.vector.tensor_copy(dst, src)
                    elif r == 1:
                        nc.scalar.copy(dst, src)
                    else:
                        nc.gpsimd.tensor_copy(dst, src)
                    if prev_pvst:
                        emit_pv_chunks(prev_pvst, g * pv_per_gap, (g + 1) * pv_per_gap)
                    c += jn
                    g += 1
                if prev_pvst:
                    emit_pv_chunks(prev_pvst, g * pv_per_gap, prev_nch)
                pvq = accp.tile([128, 65], f32, tag="acc", name="pvq")
                return (row, ptT, pvq)

            def emit_otT(sb_i, nout_t, ot_t):
                for hp in range(NPAIR):
                    for h in range(2):
                        otp = accp.tile([64, SB], bf16, tag="acc", name="otp")
                        for j in range(4):
                            nc.tensor.matmul(
                                otp[:, bass.ts(j, 128)],
                                nout_t[hp][:, h, j, :], identb[:],
                                is_transpose=True, start=(j == 0), stop=(j == 3))
                        nc.vector.tensor_copy(
                            ot_t[hp][bass.ds(h * 64, 64), :], otp[:])

            def emit_oproj_item(sb_i, mo, ot_t, ob):
                pso = accp.tile([128, SB], f32, tag="acc", name="pso")
                for fc in range(4):
                    nc.tensor.matmul(
                        pso[:], wo_b[:, fc, bass.ts(mo, 128)],
                        ot_t[fc][:, :], start=(fc == 0), stop=(fc == 3))
                if mo % 2 == 0:
                    nc.vector.tensor_copy(ob[:, mo, :], pso[:])
                else:
                    nc.scalar.copy(ob[:, mo, :], pso[:])

            def emit_out_dma(sb_i, ob):
                nc.sync.dma_start(
                    oT_d[:, bass.ts(sb_i, SB)].rearrange("(a p) b -> p a b", p=128),
                    ob[:])

            # ---------- main pipelined schedule ----------
            prev = None          # (sb_i, ot_t, ob) awaiting o-proj
            pvst = None          # PV-state of the row awaiting its PV chunks
            pvst_nout = None
            for sb_i in range(NSB):
                emit_proj_pass(sb_i, 0)
                emit_proj_pass(sb_i, 1)

                # oproj filler items for previous block, woven between rows
                filler = []
                if prev is not None:
                    psb, pot, pob = prev
                    filler = [(psb, mo, pot, pob) for mo in range(16)]
                fi = 0

                nout_t = [nopool.tile([128, 2, 4, 64], bf16, tag=f"no{hp}",
                                      name=f"no{hp}") for hp in range(NPAIR)]
                ot_t = [otpool.tile([128, SB], bf16, tag=f"ot{hp}", name=f"ot{hp}")
                        for hp in range(NPAIR)]

                rows = [(hp, h, sb_i * 4 + j)
                        for j in range(4) for hp in range(NPAIR) for h in range(2)]
                pending = []
                for ri, row in enumerate(rows):
                    p_t = emit_scores(row)
                    pending.append((row, p_t))
                    # weave o-proj of prev block: 16 items over 32 rows
                    if fi < len(filler) and ri % 2 == 1:
                        emit_oproj_item(*filler[fi])
                        fi += 1
                    if len(pending) > ROW_LAG:
                        prow, pp_t = pending.pop(0)
                        pvst_new = emit_transpose(prow, pp_t, pvst)
                        if pvst is not None:
                            emit_pv_finish(pvst, pvst_nout)
                        pvst, pvst_nout = pvst_new, nout_t
                while pending:
                    prow, pp_t = pending.pop(0)
                    pvst_new = emit_transpose(prow, pp_t, pvst)
                    if pvst is not None:
                        emit_pv_finish(pvst, pvst_nout)
                    pvst, pvst_nout = pvst_new, nout_t
                # drain the last row's PV before otT needs it
                emit_pv_chunks(pvst, 0, pvst[0][2] + 1)
                emit_pv_finish(pvst, pvst_nout)
                pvst = None
                while fi < len(filler):
                    emit_oproj_item(*filler[fi])
                    fi += 1
                if prev is not None:
                    emit_out_dma(prev[0], prev[2])

                emit_otT(sb_i, nout_t, ot_t)
                ob = obp.tile([128, 16, SB], bf16, tag="ob")
                prev = (sb_i, ot_t, ob)

            # tail: o-proj for the last block
            psb, pot, pob = prev
            for mo in range(16):
                emit_oproj_item(psb, mo, pot, pob)
            emit_out_dma(psb, pob)

    nc.compile()
    return nc


_PROG = None


def kernel(x, wq, wk, wv, wo):
    global _PROG
    if _PROG is None:
        _PROG = build_program()
    nc = _PROG

    twq = _ternarize(wq) / 8.0          # fold softmax scale into q
    twk = _ternarize(wk)
    twv = _ternarize(wv)
    two = _ternarize(wo)
    tri_np = (np.triu(np.ones((128, 128), np.float64), 1) * -1e30).astype(np.float32)

    xT = [np.ascontiguousarray(x[b].astype(np.float32).T) for b in range(BSZ)]
    in_maps = []
    for c in range(8):
        b, hq = c % 2, c // 2
        qcols = slice(hq * 512, (hq + 1) * 512)
        kvcols = slice(hq * 128, (hq + 1) * 128)
        in_maps.append({
            "xT": xT[b],
            "wq": np.ascontiguousarray(twq.T[:, qcols]).astype(np.float32),
            "wk": np.ascontiguousarray(twk.T[:, kvcols]).astype(np.float32),
            "wv": np.ascontiguousarray(twv.T[:, kvcols]).astype(np.float32),
            "wo": np.ascontiguousarray(two.T[hq * 512:(hq + 1) * 512, :]).astype(bf),
            "tri": tri_np,
        })

    res = run_bass_kernel_spmd(nc, in_maps, list(range(8)))

    out = np.zeros((BSZ, SEQ, DIM), np.float32)
    for c in range(8):
        b = c % 2
        out[b] += res.results[c]["oT"].astype(np.float32).T
    return out
